# revision 9
# baseline (speedup 1.0000x reference)
"""Trainium2 Bass kernel for nn_Attention_63007170232645 (sparse_attention).

Computation (per batch b):
    ms = BN_phi(s_b) @ W_phi.T          (256, 2048)
    mh = BN_psi(h_b) @ W_psi.T          (1024, 512)
    e[h] = ms_h @ mh.T                  (4, 256, 1024)
    a = softmax(e, -1)                  -> output (4, B, 256, 1024)
    ctx[h] = a[h] @ h_b                 (256, 4*512)
    c = BN_red(ctx) @ W_red.T           (256, 512)  -> output (B, 256, 512)

Strategy: data-parallel over batch B=32 across 8 cores (4 batches/core, no
collectives). BN is eval-mode affine, folded into the following Linear on
host. Activations/weights feed the PE in bf16 (fp32 PSUM accumulation);
feature-major ("transposed") layouts are host-prepared so no activation
transposes are needed except the attention matrix itself, which goes through
the DMA xbar transpose (2-byte dtype).
"""

import numpy as np
import ml_dtypes

import concourse.bass as bass
import concourse.bacc as bacc
import concourse.tile as tile
from concourse import mybir
from concourse.bass_utils import run_bass_kernel_spmd

BF16 = mybir.dt.bfloat16
F32 = mybir.dt.float32

NCORES = 8
B, Tq, Tk, D = 32, 256, 1024, 512
H, PROJ = 4, 512
BPC = B // NCORES  # batches per core
BN_EPS = 1e-5

P = 128  # partitions


def build_nc(reps=1):
    nc = bacc.Bacc()

    # Per-core DRAM parameters (shard shapes).
    sT_d = nc.declare_dram_parameter("sT", [BPC, D, Tq], BF16, isOutput=False)
    h_d = nc.declare_dram_parameter("h", [BPC, Tk, D], BF16, isOutput=False)
    hT_d = nc.declare_dram_parameter("hT", [BPC, D, Tk], BF16, isOutput=False)
    wphi_d = nc.declare_dram_parameter("wphiT", [D, H * PROJ], BF16, isOutput=False)
    wpsi_d = nc.declare_dram_parameter("wpsiT", [D, PROJ], BF16, isOutput=False)
    wred_d = nc.declare_dram_parameter("wredT", [H * D, D], BF16, isOutput=False)
    bphi_d = nc.declare_dram_parameter("bphi", [P, 16], F32, isOutput=False)
    bpsi_d = nc.declare_dram_parameter("bpsi", [P, 4], F32, isOutput=False)
    bred_d = nc.declare_dram_parameter("bredB", [P, D], F32, isOutput=False)
    a_d = nc.declare_dram_parameter("a", [H, BPC, Tq, Tk], F32, isOutput=True)
    c_d = nc.declare_dram_parameter("c", [BPC, Tq, D], F32, isOutput=True)

    with tile.TileContext(nc) as tc:
        with (
            tc.tile_pool(name="weights", bufs=1) as wpool,
            tc.tile_pool(name="acts", bufs=2) as apool,
            tc.tile_pool(name="mid", bufs=2) as mpool,
            tc.tile_pool(name="soft", bufs=3) as spool,
            tc.tile_pool(name="at", bufs=6) as atpool,
            tc.tile_pool(name="outs", bufs=3) as opool,
            tc.tile_pool(name="pse", bufs=2, space="PSUM") as pse,
            tc.tile_pool(name="psm", bufs=2, space="PSUM") as psm,
            tc.tile_pool(name="psc", bufs=2, space="PSUM") as psc,
        ):
            # ---- load weights / biases once ----
            wphi = wpool.tile([P, 4, H * PROJ], BF16)  # [p, kc, m] hd=kc*128+p
            nc.sync.dma_start(
                out=wphi, in_=wphi_d[:, :].rearrange("(kc p) m -> p kc m", p=P)
            )
            wpsi = wpool.tile([P, 4, PROJ], BF16)
            nc.sync.dma_start(
                out=wpsi, in_=wpsi_d[:, :].rearrange("(kc p) m -> p kc m", p=P)
            )
            wred = wpool.tile([P, 16, D], BF16)
            nc.sync.dma_start(
                out=wred, in_=wred_d[:, :].rearrange("(kc p) m -> p kc m", p=P)
            )
            bphi = wpool.tile([P, 16], F32)
            nc.sync.dma_start(out=bphi, in_=bphi_d[:, :])
            bpsi = wpool.tile([P, 4], F32)
            nc.sync.dma_start(out=bpsi, in_=bpsi_d[:, :])
            bred = wpool.tile([P, D], F32)
            nc.sync.dma_start(out=bred, in_=bred_d[:, :])

            for b in [bb for _ in range(reps) for bb in range(BPC)]:
                # ---- load activations (feature-major) ----
                sT_t = apool.tile([P, 4, Tq], BF16)  # d = kc*128+p
                nc.sync.dma_start(
                    out=sT_t, in_=sT_d[b].rearrange("(kc p) q -> p kc q", p=P)
                )
                hT_t = apool.tile([P, 4, Tk], BF16)
                nc.sync.dma_start(
                    out=hT_t, in_=hT_d[b].rearrange("(kc p) k -> p kc k", p=P)
                )
                h_t = apool.tile([P, 8, D], BF16)  # k = kt*128+p
                nc.sync.dma_start(
                    out=h_t, in_=h_d[b].rearrange("(kt p) v -> p kt v", p=P)
                )

                # ---- msT = W_phi' @ s^T + b_phi : [2048, 256] as [p, m, q] ----
                msT = mpool.tile([P, 16, Tq], BF16)
                for m in range(16):
                    ps = psm.tile([P, Tq], F32)
                    for kc in range(4):
                        nc.tensor.matmul(
                            ps,
                            lhsT=wphi[:, kc, m * P : (m + 1) * P],
                            rhs=sT_t[:, kc, :],
                            start=(kc == 0),
                            stop=(kc == 3),
                        )
                    nc.scalar.activation(
                        msT[:, m, :],
                        ps,
                        mybir.ActivationFunctionType.Identity,
                        bias=bphi[:, m : m + 1],
                    )

                # ---- mhT = W_psi' @ h^T + b_psi : [512, 1024] as [p, m, k] ----
                mhT = mpool.tile([P, 4, Tk], BF16)
                for m in range(4):
                    for nch in range(2):
                        ps = psm.tile([P, 512], F32)
                        for kc in range(4):
                            nc.tensor.matmul(
                                ps,
                                lhsT=wpsi[:, kc, m * P : (m + 1) * P],
                                rhs=hT_t[:, kc, nch * 512 : (nch + 1) * 512],
                                start=(kc == 0),
                                stop=(kc == 3),
                            )
                        nc.scalar.activation(
                            mhT[:, m, nch * 512 : (nch + 1) * 512],
                            ps,
                            mybir.ActivationFunctionType.Identity,
                            bias=bpsi[:, m : m + 1],
                        )

                # ---- per head: scores, softmax, transpose ----
                aT = {}
                for h in range(H):
                    aT[h] = atpool.tile([P, 8, Tq], BF16, name="aT")  # [ko, kt, q]
                    for qt in range(2):
                        psE = pse.tile([P, Tk], F32)
                        for kc in range(4):
                            lhsT = msT[:, h * 4 + kc, qt * P : (qt + 1) * P]
                            for nch in range(2):
                                nc.tensor.matmul(
                                    psE[:, nch * 512 : (nch + 1) * 512],
                                    lhsT=lhsT,
                                    rhs=mhT[:, kc, nch * 512 : (nch + 1) * 512],
                                    start=(kc == 0),
                                    stop=(kc == 3),
                                )
                        # softmax over free dim (no max subtraction needed:
                        # |e| <~ 30, exp stays in fp32 range)
                        pexp = spool.tile([P, Tk], F32)
                        sums = spool.tile([P, 1], F32)
                        nc.scalar.activation(
                            pexp,
                            psE,
                            mybir.ActivationFunctionType.Exp,
                            accum_out=sums,
                        )
                        recip = spool.tile([P, 1], F32)
                        nc.vector.reciprocal(recip, sums)
                        a_f32 = opool.tile([P, Tk], F32)
                        nc.vector.tensor_scalar_mul(a_f32, in0=pexp, scalar1=recip)
                        nc.sync.dma_start(
                            out=a_d[h, b, qt * P : (qt + 1) * P, :], in_=a_f32
                        )
                        a_bf16 = spool.tile([P, Tk], BF16)
                        nc.scalar.mul(a_bf16, pexp, recip)
                        # xbar transpose [q, k] -> [ko, kt, q-chunk]
                        nc.sync.dma_start_transpose(
                            out=aT[h][:, :, qt * P : (qt + 1) * P],
                            in_=a_bf16,
                        )

                # ---- ctxT = h^T @ a^T : [2048, 256] as [p, h*4+vt, q] ----
                ctxT = mpool.tile([P, 16, Tq], BF16)
                for vt in range(4):
                    for hg in range(2):
                        pcs = [psc.tile([P, Tq], F32, name="pcs") for _ in range(2)]
                        for kt in range(8):
                            lhsT = h_t[:, kt, vt * P : (vt + 1) * P]
                            for hh in range(2):
                                h_idx = hg * 2 + hh
                                nc.tensor.matmul(
                                    pcs[hh],
                                    lhsT=lhsT,
                                    rhs=aT[h_idx][:, kt, :],
                                    start=(kt == 0),
                                    stop=(kt == 7),
                                )
                        for hh in range(2):
                            h_idx = hg * 2 + hh
                            nc.scalar.activation(
                                ctxT[:, h_idx * 4 + vt, :],
                                pcs[hh],
                                mybir.ActivationFunctionType.Copy,
                            )

                # ---- c = ctx' @ W_red'.T + b_red : [256, 512] ----
                for qt in range(2):
                    psR = psm.tile([P, D], F32, name="ps")
                    for m in range(16):
                        nc.tensor.matmul(
                            psR,
                            lhsT=ctxT[:, m, qt * P : (qt + 1) * P],
                            rhs=wred[:, m, :],
                            start=(m == 0),
                            stop=(m == 15),
                        )
                    c_sb = opool.tile([P, D], F32)
                    nc.vector.tensor_add(c_sb, psR, bred)
                    nc.sync.dma_start(
                        out=c_d[b, qt * P : (qt + 1) * P, :], in_=c_sb
                    )

    nc.finalize()
    return nc


def _prep_in_maps(inputs):
    f64 = np.float64
    s = np.asarray(inputs["s"], np.float32)
    h = np.asarray(inputs["h"], np.float32)

    def fold(gamma, beta, mean, var, W):
        scale = np.asarray(gamma, f64) / np.sqrt(np.asarray(var, f64) + BN_EPS)
        shift = np.asarray(beta, f64) - np.asarray(mean, f64) * scale
        Wd = np.asarray(W, f64)
        W_eff = Wd * scale[None, :]  # (out, in)
        b_eff = Wd @ shift  # (out,)
        return W_eff, b_eff

    Wphi_eff, bphi = fold(
        inputs["phi_gamma"], inputs["phi_beta"], inputs["phi_mean"],
        inputs["phi_var"], inputs["W_phi"],
    )
    Wpsi_eff, bpsi = fold(
        inputs["psi_gamma"], inputs["psi_beta"], inputs["psi_mean"],
        inputs["psi_var"], inputs["W_psi"],
    )
    Wred_eff, bred = fold(
        inputs["red_gamma"], inputs["red_beta"], inputs["red_mean"],
        inputs["red_var"], inputs["W_red"],
    )

    bf = ml_dtypes.bfloat16
    wphiT = np.ascontiguousarray(Wphi_eff.T.astype(np.float32)).astype(bf)  # (512, 2048)
    wpsiT = np.ascontiguousarray(Wpsi_eff.T.astype(np.float32)).astype(bf)  # (512, 512)
    wredT = np.ascontiguousarray(Wred_eff.T.astype(np.float32)).astype(bf)  # (2048, 512)
    bphi_t = np.ascontiguousarray(
        bphi.astype(np.float32).reshape(16, P).T
    )  # (128, 16)
    bpsi_t = np.ascontiguousarray(bpsi.astype(np.float32).reshape(4, P).T)  # (128, 4)
    bredB = np.ascontiguousarray(
        np.tile(bred.astype(np.float32)[None, :], (P, 1))
    )  # (128, 512)

    in_maps = []
    for ci in range(NCORES):
        sl = slice(ci * BPC, (ci + 1) * BPC)
        s_b = s[sl]
        h_b = h[sl]
        in_maps.append(
            {
                "sT": np.ascontiguousarray(s_b.transpose(0, 2, 1)).astype(bf),
                "h": np.ascontiguousarray(h_b).astype(bf),
                "hT": np.ascontiguousarray(h_b.transpose(0, 2, 1)).astype(bf),
                "wphiT": wphiT,
                "wpsiT": wpsiT,
                "wredT": wredT,
                "bphi": bphi_t,
                "bpsi": bpsi_t,
                "bredB": bredB,
            }
        )
    return in_maps


def _run(inputs, trace=False, **kw):
    in_maps = _prep_in_maps(inputs)
    nc = build_nc()
    res = run_bass_kernel_spmd(
        nc, in_maps, core_ids=list(range(NCORES)), trace=trace, **kw
    )
    a_full = np.concatenate([r["a"] for r in res.results], axis=1)
    c_full = np.concatenate([r["c"] for r in res.results], axis=0)
    return (a_full.astype(np.float32), c_full.astype(np.float32)), res


def kernel(**inputs):
    out, _ = _run(inputs, trace=False)
    return out


if __name__ == "__main__":
    rng = np.random.default_rng(0)
    fake = {
        "s": rng.standard_normal((B, Tq, D), np.float32),
        "h": rng.standard_normal((B, Tk, D), np.float32),
        "phi_gamma": np.ones(D, np.float32),
        "phi_beta": np.zeros(D, np.float32),
        "phi_mean": rng.standard_normal(D).astype(np.float32) * 0.1,
        "phi_var": rng.uniform(0.5, 1.5, D).astype(np.float32),
        "W_phi": (rng.standard_normal((PROJ * H, D)) * 0.02).astype(np.float32),
        "psi_gamma": np.ones(D, np.float32),
        "psi_beta": np.zeros(D, np.float32),
        "psi_mean": rng.standard_normal(D).astype(np.float32) * 0.1,
        "psi_var": rng.uniform(0.5, 1.5, D).astype(np.float32),
        "W_psi": (rng.standard_normal((PROJ, D)) * 0.02).astype(np.float32),
        "red_gamma": np.ones(D * H, np.float32),
        "red_beta": np.zeros(D * H, np.float32),
        "red_mean": rng.standard_normal(D * H).astype(np.float32) * 0.1,
        "red_var": rng.uniform(0.5, 1.5, D * H).astype(np.float32),
        "W_red": (rng.standard_normal((D, D * H)) * 0.02).astype(np.float32),
    }
    (a, c), _ = _run(fake)
    print("a", a.shape, a.dtype, "c", c.shape, c.dtype)


# revision 51
# speedup vs baseline: 3.2990x; 3.2990x over previous
"""Trainium2 Bass kernel for nn_Attention_63007170232645 (sparse_attention).

Computation (per batch b):
    ms = BN_phi(s_b) @ W_phi.T          (256, 2048)
    mh = BN_psi(h_b) @ W_psi.T          (1024, 512)
    e[h] = ms_h @ mh.T                  (4, 256, 1024)
    a = softmax(e, -1)                  -> output (4, B, 256, 1024)
    ctx[h] = a[h] @ h_b                 (256, 4*512)
    c = BN_red(ctx) @ W_red.T           (256, 512)  -> output (B, 256, 512)

Strategy: data-parallel over batch B=32 across 8 cores (4 batches/core, no
collectives). BN is eval-mode affine, folded into the following Linear on
host. Activations/weights feed the PE in bf16 (fp32 PSUM accumulation);
feature-major ("transposed") layouts are host-prepared so no activation
transposes are needed except the attention matrix itself, which goes through
the DMA xbar transpose (2-byte dtype).
"""

import numpy as np
import ml_dtypes

import concourse.bass as bass
import concourse.bacc as bacc
import concourse.tile as tile
from concourse import mybir
from concourse.bass_utils import run_bass_kernel_spmd

BF16 = mybir.dt.bfloat16
F32 = mybir.dt.float32

NCORES = 8
B, Tq, Tk, D = 32, 256, 1024, 512
H, PROJ = 4, 512
BPC = B // NCORES  # batches per core
BN_EPS = 1e-5

P = 128  # partitions


def build_nc(reps=1):
    nc = bacc.Bacc()

    # Per-core DRAM parameters (shard shapes).
    sT_d = nc.declare_dram_parameter("sT", [BPC, D, Tq], BF16, isOutput=False)
    h_d = nc.declare_dram_parameter("h", [BPC, Tk, D], BF16, isOutput=False)
    hT_d = nc.declare_dram_parameter("hT", [BPC, D, Tk], BF16, isOutput=False)
    wphi_d = nc.declare_dram_parameter("wphiT", [D, H * PROJ], BF16, isOutput=False)
    wpsi_d = nc.declare_dram_parameter("wpsiT", [D, PROJ], BF16, isOutput=False)
    wred_d = nc.declare_dram_parameter("wredT", [H * D, D], BF16, isOutput=False)
    bphi_d = nc.declare_dram_parameter("bphi", [P, 16], F32, isOutput=False)
    bpsi_d = nc.declare_dram_parameter("bpsi", [P, 4], F32, isOutput=False)
    bred_d = nc.declare_dram_parameter("bredB", [P, D], F32, isOutput=False)
    a_d = nc.declare_dram_parameter("a", [H, BPC, Tq, Tk], F32, isOutput=True)
    c_d = nc.declare_dram_parameter("c", [BPC, Tq, D], F32, isOutput=True)

    with tile.TileContext(nc) as tc:
        with (
            tc.tile_pool(name="weights", bufs=1) as wpool,
            tc.tile_pool(name="acts", bufs=2) as apool,
            tc.tile_pool(name="mid", bufs=2) as mpool,
            tc.tile_pool(name="mid3", bufs=3) as m3pool,
            tc.tile_pool(name="soft", bufs=4) as spool,
            tc.tile_pool(name="xp", bufs=8) as xpool,
            tc.tile_pool(name="at", bufs=3) as atpool,
            tc.tile_pool(name="outs", bufs=4) as opool,
            tc.tile_pool(name="pse", bufs=4, space="PSUM") as pse,
            tc.tile_pool(name="psm", bufs=2, space="PSUM") as psm,
            tc.tile_pool(name="psc", bufs=2, space="PSUM") as psc,
        ):
            # ---- load weights / biases once ----
            wphi = wpool.tile([P, 4, H * PROJ], BF16)  # [p, kc, m] hd=kc*128+p
            nc.sync.dma_start(
                out=wphi, in_=wphi_d[:, :].rearrange("(kc p) m -> p kc m", p=P)
            )
            wpsi = wpool.tile([P, 4, PROJ], BF16)
            nc.sync.dma_start(
                out=wpsi, in_=wpsi_d[:, :].rearrange("(kc p) m -> p kc m", p=P)
            )
            bphi = wpool.tile([P, 16], F32)
            nc.sync.dma_start(out=bphi, in_=bphi_d[:, :])
            bpsi = wpool.tile([P, 4], F32)
            nc.sync.dma_start(out=bpsi, in_=bpsi_d[:, :])
            # wred/bred are loaded after the first batch's activations (they
            # are only needed by attn(0)); keeps the first phi matmuls fed.
            wred = wpool.tile([P, 16, D], BF16)
            bred = wpool.tile([P, D], F32)

            _wred_loaded = [False]

            def prep(b):
                """Load + phi/psi projections for batch b -> (msT, mhT, h_t)."""
                sT_t = apool.tile([P, 4, Tq], BF16, name="sT_t")  # d = kc*128+p
                nc.gpsimd.dma_start(
                    out=sT_t, in_=sT_d[b].rearrange("(kc p) q -> p kc q", p=P)
                )
                hT_t = apool.tile([P, 4, Tk], BF16, name="hT_t")
                nc.gpsimd.dma_start(
                    out=hT_t, in_=hT_d[b].rearrange("(kc p) k -> p kc k", p=P)
                )
                h_t = apool.tile([P, 8, D], BF16, name="h_t")  # k = kt*128+p
                nc.gpsimd.dma_start(
                    out=h_t, in_=h_d[b].rearrange("(kt p) v -> p kt v", p=P)
                )
                if not _wred_loaded[0]:
                    _wred_loaded[0] = True
                    nc.gpsimd.dma_start(
                        out=wred,
                        in_=wred_d[:, :].rearrange("(kc p) m -> p kc m", p=P),
                    )
                    nc.gpsimd.dma_start(out=bred, in_=bred_d[:, :])

                # msT = W_phi' @ s^T + b_phi : [2048, 256] as [p, m, q]
                msT = m3pool.tile([P, 16, Tq], BF16, name="msT")
                for m in range(16):
                    ps = psm.tile([P, Tq], F32, name="ps")
                    for kc in range(4):
                        nc.tensor.matmul(
                            ps,
                            lhsT=wphi[:, kc, m * P : (m + 1) * P],
                            rhs=sT_t[:, kc, :],
                            start=(kc == 0),
                            stop=(kc == 3),
                        )
                    nc.vector.tensor_scalar_add(
                        msT[:, m, :], in0=ps, scalar1=bphi[:, m : m + 1]
                    )

                # mhT = W_psi' @ h^T + b_psi : [512, 1024] as [p, m, k]
                mhT = m3pool.tile([P, 4, Tk], BF16, name="mhT")
                for m in range(4):
                    for nch in range(2):
                        ps = psm.tile([P, 512], F32, name="ps")
                        for kc in range(4):
                            nc.tensor.matmul(
                                ps,
                                lhsT=wpsi[:, kc, m * P : (m + 1) * P],
                                rhs=hT_t[:, kc, nch * 512 : (nch + 1) * 512],
                                start=(kc == 0),
                                stop=(kc == 3),
                            )
                        nc.vector.tensor_scalar_add(
                            mhT[:, m, nch * 512 : (nch + 1) * 512],
                            in0=ps,
                            scalar1=bpsi[:, m : m + 1],
                        )
                return msT, mhT, h_t

            def attn(b, msT, mhT, h_t):
                """Scores/softmax/ctx/reduce for batch b."""
                # ctxT[p, vt, hg, hh*256+q] ; hv = (2*hg+hh)*512 + vt*128 + p
                ctxT = mpool.tile([P, 4, 2, 2 * Tq], BF16, name="ctxT")
                deferred = []  # (h, qt, pexp, recip) -> a_f32 after red
                for hg in range(2):
                    # aT2: transposed attention, head-pair packed:
                    # [ko, kt, hh*256 + q]
                    aT2 = atpool.tile([P, 8, 2 * Tq], BF16, name="aT")
                    for hh in range(2):
                        h = 2 * hg + hh
                        for qt in range(2):
                            pexp = xpool.tile([P, Tk], BF16, name="pexp")
                            ssum = spool.tile([P, 2], F32, name="ssum")
                            for nch in range(2):
                                psE = pse.tile([P, 512], F32, name="psE")
                                for kc in range(4):
                                    nc.tensor.matmul(
                                        psE,
                                        lhsT=msT[
                                            :, h * 4 + kc, qt * P : (qt + 1) * P
                                        ],
                                        rhs=mhT[:, kc, nch * 512 : (nch + 1) * 512],
                                        start=(kc == 0),
                                        stop=(kc == 3),
                                    )
                                # softmax over free dim (no max subtraction:
                                # |e| <~ 30, exp stays in fp32 range)
                                nc.scalar.activation(
                                    pexp[:, nch * 512 : (nch + 1) * 512],
                                    psE,
                                    mybir.ActivationFunctionType.Exp,
                                    accum_out=ssum[:, nch : nch + 1],
                                )
                            sums = spool.tile([P, 1], F32, name="sums")
                            nc.vector.reduce_sum(
                                sums, ssum, axis=mybir.AxisListType.X
                            )
                            recip = xpool.tile([P, 1], F32, name="recip")
                            nc.vector.reciprocal(recip, sums)
                            a_bf16 = spool.tile([P, Tk], BF16, name="a_bf16")
                            nc.vector.tensor_scalar_mul(
                                a_bf16, in0=pexp, scalar1=recip
                            )
                            # xbar transpose [q, k] -> [ko, kt, q-chunk]
                            nc.sync.dma_start_transpose(
                                out=aT2[
                                    :, :, hh * Tq + qt * P : hh * Tq + (qt + 1) * P
                                ],
                                in_=a_bf16,
                            )
                            deferred.append((h, qt, pexp, recip))

                    # ctxT for this head pair
                    for vt in range(4):
                        pc = psc.tile([P, 2 * Tq], F32, name="pc")
                        for kt in range(8):
                            nc.tensor.matmul(
                                pc,
                                lhsT=h_t[:, kt, vt * P : (vt + 1) * P],
                                rhs=aT2[:, kt, :],
                                start=(kt == 0),
                                stop=(kt == 7),
                            )
                        nc.scalar.activation(
                            ctxT[:, vt, hg, :],
                            pc,
                            mybir.ActivationFunctionType.Copy,
                        )

                # c = ctx' @ W_red'.T + b_red : [256, 512]
                for qt in range(2):
                    psR = psm.tile([P, D], F32, name="ps")
                    for m in range(16):
                        hg, hh, vt = m // 8, (m // 4) % 2, m % 4
                        lhsT = ctxT[
                            :, vt, hg, hh * Tq + qt * P : hh * Tq + (qt + 1) * P
                        ]
                        nc.tensor.matmul(
                            psR,
                            lhsT=lhsT,
                            rhs=wred[:, (2 * hg + hh) * 4 + vt, :],
                            start=(m == 0),
                            stop=(m == 15),
                        )
                    c_sb = opool.tile([P, D], F32, name="c_sb")
                    nc.vector.tensor_add(c_sb, psR, bred)
                    nc.gpsimd.dma_start(
                        out=c_d[b, qt * P : (qt + 1) * P, :], in_=c_sb
                    )

                # deferred a_f32 normalize + store (fully off critical path)
                for h, qt, pexp, recip in deferred:
                    a_f32 = opool.tile([P, Tk], F32, name="a_f32")
                    nc.scalar.mul(a_f32, pexp, recip)
                    nc.scalar.dma_start(
                        out=a_d[h, b, qt * P : (qt + 1) * P, :], in_=a_f32
                    )

            # software pipeline: two preps ahead of each attn
            order = [bb for _ in range(reps) for bb in range(BPC)]
            from collections import deque
            q = deque()
            DEPTH = 3
            for b in order:
                q.append((b, *prep(b)))
                if len(q) > DEPTH:
                    a0 = q.popleft()
                    attn(a0[0], *a0[1:])
            while q:
                a0 = q.popleft()
                attn(a0[0], *a0[1:])

    nc.finalize()
    return nc


def _prep_in_maps(inputs):
    f64 = np.float64
    s = np.asarray(inputs["s"], np.float32)
    h = np.asarray(inputs["h"], np.float32)

    def fold(gamma, beta, mean, var, W):
        scale = np.asarray(gamma, f64) / np.sqrt(np.asarray(var, f64) + BN_EPS)
        shift = np.asarray(beta, f64) - np.asarray(mean, f64) * scale
        Wd = np.asarray(W, f64)
        W_eff = Wd * scale[None, :]  # (out, in)
        b_eff = Wd @ shift  # (out,)
        return W_eff, b_eff

    Wphi_eff, bphi = fold(
        inputs["phi_gamma"], inputs["phi_beta"], inputs["phi_mean"],
        inputs["phi_var"], inputs["W_phi"],
    )
    Wpsi_eff, bpsi = fold(
        inputs["psi_gamma"], inputs["psi_beta"], inputs["psi_mean"],
        inputs["psi_var"], inputs["W_psi"],
    )
    Wred_eff, bred = fold(
        inputs["red_gamma"], inputs["red_beta"], inputs["red_mean"],
        inputs["red_var"], inputs["W_red"],
    )

    bf = ml_dtypes.bfloat16
    wphiT = np.ascontiguousarray(Wphi_eff.T.astype(np.float32)).astype(bf)  # (512, 2048)
    wpsiT = np.ascontiguousarray(Wpsi_eff.T.astype(np.float32)).astype(bf)  # (512, 512)
    wredT = np.ascontiguousarray(Wred_eff.T.astype(np.float32)).astype(bf)  # (2048, 512)
    bphi_t = np.ascontiguousarray(
        bphi.astype(np.float32).reshape(16, P).T
    )  # (128, 16)
    bpsi_t = np.ascontiguousarray(bpsi.astype(np.float32).reshape(4, P).T)  # (128, 4)
    bredB = np.ascontiguousarray(
        np.tile(bred.astype(np.float32)[None, :], (P, 1))
    )  # (128, 512)

    in_maps = []
    for ci in range(NCORES):
        sl = slice(ci * BPC, (ci + 1) * BPC)
        s_b = s[sl]
        h_b = h[sl]
        in_maps.append(
            {
                "sT": np.ascontiguousarray(s_b.transpose(0, 2, 1)).astype(bf),
                "h": np.ascontiguousarray(h_b).astype(bf),
                "hT": np.ascontiguousarray(h_b.transpose(0, 2, 1)).astype(bf),
                "wphiT": wphiT,
                "wpsiT": wpsiT,
                "wredT": wredT,
                "bphi": bphi_t,
                "bpsi": bpsi_t,
                "bredB": bredB,
            }
        )
    return in_maps


def _run(inputs, trace=False, **kw):
    in_maps = _prep_in_maps(inputs)
    nc = build_nc()
    res = run_bass_kernel_spmd(
        nc, in_maps, core_ids=list(range(NCORES)), trace=trace, **kw
    )
    a_full = np.concatenate([r["a"] for r in res.results], axis=1)
    c_full = np.concatenate([r["c"] for r in res.results], axis=0)
    return (a_full.astype(np.float32), c_full.astype(np.float32)), res


def kernel(**inputs):
    out, _ = _run(inputs, trace=False)
    return out


if __name__ == "__main__":
    rng = np.random.default_rng(0)
    fake = {
        "s": rng.standard_normal((B, Tq, D), np.float32),
        "h": rng.standard_normal((B, Tk, D), np.float32),
        "phi_gamma": np.ones(D, np.float32),
        "phi_beta": np.zeros(D, np.float32),
        "phi_mean": rng.standard_normal(D).astype(np.float32) * 0.1,
        "phi_var": rng.uniform(0.5, 1.5, D).astype(np.float32),
        "W_phi": (rng.standard_normal((PROJ * H, D)) * 0.02).astype(np.float32),
        "psi_gamma": np.ones(D, np.float32),
        "psi_beta": np.zeros(D, np.float32),
        "psi_mean": rng.standard_normal(D).astype(np.float32) * 0.1,
        "psi_var": rng.uniform(0.5, 1.5, D).astype(np.float32),
        "W_psi": (rng.standard_normal((PROJ, D)) * 0.02).astype(np.float32),
        "red_gamma": np.ones(D * H, np.float32),
        "red_beta": np.zeros(D * H, np.float32),
        "red_mean": rng.standard_normal(D * H).astype(np.float32) * 0.1,
        "red_var": rng.uniform(0.5, 1.5, D * H).astype(np.float32),
        "W_red": (rng.standard_normal((D, D * H)) * 0.02).astype(np.float32),
    }
    (a, c), _ = _run(fake)
    print("a", a.shape, a.dtype, "c", c.shape, c.dtype)
